# revision 28
# baseline (speedup 1.0000x reference)
"""AttentionBlock Trainium2 kernel: 8-way batch-parallel over 8 NeuronCores.

Reference computation (per batch element b):
    tokens = x[b].reshape(C, N).T                  # [N, C], N=1024, C=512
    qkv    = tokens @ w_proj + b_proj              # [N, 3*512]
    per head h (8 heads, D=64):
        att  = softmax(q_h @ k_h.T / 8, axis=keys) # [N, N]
        res_h = att @ v_h                          # [N, 64]
    out = res @ w_out + b_out + tokens             # [N, C]
    return out.T.reshape(C, 32, 32)

Kernel strategy (per core, one batch element):
  - qk projection computed transposed: qkT = w_qk.T @ x  -> SBUF [d, tokens]
    (w_proj columns host-permuted so each head-pair's q/k occupy partition
    halves 0-63 / 64-127, enabling row-packed K=64 score matmuls), fp8
    DoubleRow matmuls with x16 weight prescale, descale fused into bias add
  - scores computed transposed scT[j, i] = k.T @ q (bf16, K=64 row-packed);
    exp on ScalarE from PSUM with the 1/8 softmax scale folded into the
    activation's scale operand, output directly in fp8e4
  - v projection (fp8 DR) with a ones column appended per head; attn@v
    fp8 DoubleRow matmul yields [d | sum] x tokens so the softmax
    denominator rides the same accumulation (M=65)
  - normalize via DVE reciprocal_approx_fast on the PSUM denominator row,
    GpSimd partition_broadcast (keeps TensorE off the normalization path),
    DVE multiply reading PSUM directly
  - out projection (fp8 DR) gives the output directly in x layout;
    residual+bias prefilled, descale fused via scalar_tensor_tensor
"""
import sys
sys.path.insert(0, '/opt/trn_rl_repo')

import numpy as np
import ml_dtypes
from contextlib import ExitStack

B, C, N = 8, 512, 1024
NH, D = 8, 64
INNER = NH * D  # 512
SCALE = D ** -0.5
W8 = 16.0  # fp8 weight prescale

bf16 = ml_dtypes.bfloat16

_cached_run = None
_cached_nc = None


# ---------------------------------------------------------------- bass kernel
def _build_nc():
    import concourse.bass as bass
    import concourse.tile as tile
    from concourse import bacc, mybir

    f32 = mybir.dt.float32
    b16 = mybir.dt.bfloat16
    f8 = mybir.dt.float8e4
    DR = mybir.MatmulPerfMode.DoubleRow
    ts = bass.ts

    nc = bacc.Bacc("TRN2", target_bir_lowering=False, debug=False)

    x_d = nc.dram_tensor("x", [C, N], f32, kind="ExternalInput").ap()
    xb_d = nc.dram_tensor("xb", [C, N], f8, kind="ExternalInput").ap()
    wqk_d = nc.dram_tensor("wqk", [C, 1024], f8, kind="ExternalInput").ap()
    bqk_d = nc.dram_tensor("bqk", [128, 8], f32, kind="ExternalInput").ap()
    wv_d = nc.dram_tensor("wv", [C, 512], f8, kind="ExternalInput").ap()
    bvb_d = nc.dram_tensor("bvb", [128, 512], f32, kind="ExternalInput").ap()
    wo_d = nc.dram_tensor("wo", [INNER, C], b16, kind="ExternalInput").ap()
    bo_d = nc.dram_tensor("bo", [128, 4], f32, kind="ExternalInput").ap()
    out_d = nc.dram_tensor("out", [C, N], f32, kind="ExternalOutput").ap()

    with tile.TileContext(nc) as tc, ExitStack() as ctx:
        sb = ctx.enter_context(tc.tile_pool(name="sb", bufs=1))
        upool = ctx.enter_context(tc.tile_pool(name="up", bufs=1))
        rpool = ctx.enter_context(tc.tile_pool(name="rp", bufs=1))

        # ---- persistent SBUF tensors (qk-proj operands first)
        xb_sb = sb.tile([128, 4, N], f8)
        nc.sync.dma_start(xb_sb[:], xb_d.rearrange("(kc p) n -> p kc n", p=128))
        wqk_sb = sb.tile([128, 4, 1024], f8)
        nc.sync.dma_start(wqk_sb[:], wqk_d.rearrange("(kc p) j -> p kc j", p=128))
        bqk_sb = sb.tile([128, 8], f32)
        nc.sync.dma_start(bqk_sb[:], bqk_d[:])
        wv_sb = sb.tile([128, 4, 512], f8)
        nc.sync.dma_start(wv_sb[:], wv_d.rearrange("(kc p) j -> p kc j", p=128))
        bvb_sb = sb.tile([128, 512], f32)
        nc.sync.dma_start(bvb_sb[:], bvb_d[:])
        x_sb = sb.tile([128, 4, N], f32)
        nc.sync.dma_start(x_sb[:], x_d.rearrange("(kc p) n -> p kc n", p=128))
        wo_sb = sb.tile([128, 4, 512], b16)
        nc.sync.dma_start(wo_sb[:], wo_d.rearrange("(kc p) c -> p kc c", p=128))
        bo_sb = sb.tile([128, 4], f32)
        nc.sync.dma_start(bo_sb[:], bo_d[:])

        qkT_sb = sb.tile([128, 8, N], b16)      # [inner%128, qk chunk, token]
        v_sb = sb.tile([128, 8, 8 * 66], b16)   # [token%128, tchunk, h*66+(d|one|pad)]
        v4 = v_sb.rearrange("p t (h w) -> p t h w", w=66)
        resT_sb = sb.tile([128, 4, N], b16)     # [inner%128, pair, token]
        final_sb = sb.tile([128, 4, N], f32)    # [c%128, cchunk, token]

        nc.vector.memset(v4[:, :, :, 64], 1.0)  # ones column per head
        nc.vector.memset(v4[:, :, :, 65], 0.0)  # pad (even M for fp8 DR)
        ones_sb = sb.tile([1, 64], b16)
        nc.vector.memset(ones_sb[:], 1.0)  # lhsT for K=1 recip partition-bcast

        # ---- attention: pair-pipelined; PE order sc(0),sc(1),av(0),sc(2),...
        def scores_pair(t, interleave=None):
            """Row-packed K=64 bf16 score matmuls + exp for head pair t;
            `interleave(jc)` emits the previous pair's attn@v chunk between
            jc groups so PE never idles on the exp ping-pong."""
            qc, kc = 2 * t, 2 * t + 1
            uA = upool.tile([128, 8, N], b16, tag="U", bufs=4, name=f"u{2*t}")
            uB = upool.tile([128, 8, N], b16, tag="U", bufs=4, name=f"u{2*t+1}")
            for jc in range(8):
                if interleave is not None:
                    interleave(jc)
                sA = scA.tile([128, 2, 512], f32, tag="scA", bufs=1, name=f"sA{t}_{jc}")
                sB = scB.tile([128, 2, 512], f32, tag="scB", bufs=1, name=f"sB{t}_{jc}")
                for ih in range(2):
                    nc.tensor.matmul(
                        sA[:, ih, :],
                        lhsT=qkT_sb[0:64, kc, ts(jc, 128)],
                        rhs=qkT_sb[0:64, qc, ts(ih, 512)],
                        start=True, stop=True)
                for ih in range(2):
                    nc.tensor.matmul(
                        sB[:, ih, :],
                        lhsT=qkT_sb[64:128, kc, ts(jc, 128)],
                        rhs=qkT_sb[64:128, qc, ts(ih, 512)],
                        start=True, stop=True)
                nc.scalar.activation(
                    uA[:, jc, :], sA.rearrange("p a b -> p (a b)"),
                    mybir.ActivationFunctionType.Exp)
                nc.scalar.activation(
                    uB[:, jc, :], sB.rearrange("p a b -> p (a b)"),
                    mybir.ActivationFunctionType.Exp)
            return uA, uB

        def attn_v_jc(t, uA, uB, pair_res, jc):
            """One jc-chunk of bf16 attn@v for both heads of pair t."""
            for half in range(2):
                h = 2 * t + half
                u = uA if half == 0 else uB
                if jc == 0:
                    pair_res.append(rsp.tile([128, 2, 512], f32, tag="res",
                                             bufs=2, name=f"res{h}"))
                res = pair_res[half]
                for ih in range(2):
                    nc.tensor.matmul(
                        res[0:65, ih, :],
                        lhsT=v4[:, jc, h, 0:65],
                        rhs=u[:, jc, ts(ih, 512)],
                        start=(jc == 0), stop=(jc == 7))

        def normalize(t, pair_res):
            """Baseline-proven normalization: copy PSUM denominator row to
            SBUF, reciprocal_approx_fast (SBUF->SBUF), bf16 cast, K=1
            ones-matmul partition broadcast into its own PSUM pool, copy to
            SBUF, multiply straight off the res PSUM."""
            for half in range(2):
                h = 2 * t + half
                res = pair_res[half]
                den = rpool.tile([1, N], f32, tag="den", bufs=2, name=f"dn{h}")
                nc.vector.tensor_copy(
                    den[:], res[64:65].rearrange("p a b -> p (a b)"))
                rcp = rpool.tile([1, N], f32, tag="rcp", bufs=2, name=f"rc{h}")
                nc.vector.reciprocal_approx_fast(rcp[:], den[:])
                rcpb = rpool.tile([1, N], b16, tag="rcpb", bufs=2,
                                  name=f"rcb{h}")
                with nc.allow_low_precision(reason="bf16 softmax recip bcast"):
                    nc.vector.tensor_copy(rcpb[:], rcp[:])
                # broadcast lands in the res tile's unused partitions
                # 64:128 (den row already copied to SBUF, so WAR is safe)
                for ih in range(2):
                    nc.tensor.matmul(
                        res[64:128, ih, :],
                        lhsT=ones_sb[:],
                        rhs=rcpb[0:1, ts(ih, 512)],
                        start=True, stop=True)
                bcs = rpool.tile([64, N], f32, tag="bcs", bufs=2,
                                 name=f"bcs{h}")
                nc.vector.tensor_copy(
                    bcs[:], res[64:128].rearrange("p a b -> p (a b)"))
                with nc.allow_low_precision(reason="bf16 attention output"):
                    if half == 0:
                        nc.vector.tensor_mul(
                            resT_sb[0:64, t, :],
                            res[0:64].rearrange("p a b -> p (a b)"),
                            bcs[:])
                    else:
                        tmp = rpool.tile([64, N], b16, tag="tmpod", bufs=2,
                                         name=f"tm{h}")
                        nc.vector.tensor_mul(
                            tmp[:],
                            res[0:64].rearrange("p a b -> p (a b)"),
                            bcs[:])
                        nc.sync.dma_start(resT_sb[64:128, t, :], tmp[:])

        with tc.tile_pool(name="scA", bufs=1, space="PSUM") as scA, \
             tc.tile_pool(name="scB", bufs=1, space="PSUM") as scB:
            # ---- projections (fp8 DoubleRow K=256); scores(0) emitted
            # between qk-proj and v-proj so ScalarE exp starts early
            with tc.tile_pool(name="pp", bufs=2, space="PSUM") as pp:
                for m in range(8):
                    ps = pp.tile([128, 2, 512], f32, tag="pp", name=f"qk{m}")
                    for kcp in range(2):
                        for ih in range(2):
                            nc.tensor.matmul(
                                ps[:, ih, :],
                                lhsT=wqk_sb[:, 2 * kcp:2 * kcp + 2, ts(m, 128)],
                                rhs=xb_sb[:, 2 * kcp:2 * kcp + 2, ts(ih, 512)],
                                start=(kcp == 0), stop=(kcp == 1), perf_mode=DR)
                    # qkT = psum/16 + bias (descale fused; q chunks also
                    # carry the 1/8 softmax scale so exp needs no scale operand)
                    nc.vector.tensor_scalar(
                        qkT_sb[:, m, :], ps.rearrange("p a b -> p (a b)"),
                        (SCALE if m % 2 == 0 else 1.0) / W8, bqk_sb[:, m, None],
                        op0=mybir.AluOpType.mult, op1=mybir.AluOpType.add)

                us = {0: scores_pair(0)}

                for c2 in range(4):
                    ps = pp.tile([128, 2, 512], f32, tag="pp", name=f"v{c2}")
                    for half in range(2):
                        tch = 2 * c2 + half
                        for kcp in range(2):
                            nc.tensor.matmul(
                                ps[:, half, :],
                                lhsT=xb_sb[:, 2 * kcp:2 * kcp + 2, ts(tch, 128)],
                                rhs=wv_sb[:, 2 * kcp:2 * kcp + 2, :],
                                start=(kcp == 0), stop=(kcp == 1), perf_mode=DR)
                    for half in range(2):
                        # v = psum/16 + bias -> bf16
                        nc.vector.scalar_tensor_tensor(
                            v4[:, 2 * c2 + half, :, 0:64],
                            ps[:, half, :].rearrange("p (h d) -> p h d", d=64),
                            1.0 / W8,
                            bvb_sb.rearrange("p (h d) -> p h d", d=64),
                            op0=mybir.AluOpType.mult, op1=mybir.AluOpType.add)

            with tc.tile_pool(name="rsp", bufs=1, space="PSUM") as rsp:
                for cc in range(4):  # final = x + b_out (residual+bias prefill)
                    nc.vector.tensor_scalar_add(
                        final_sb[:, cc, :], x_sb[:, cc, :], bo_sb[:, cc, None])

                for t in range(4):
                    pair_res = []
                    uA, uB = us.pop(t)
                    if t + 1 < 4:
                        us[t + 1] = scores_pair(
                            t + 1,
                            interleave=lambda jc: attn_v_jc(
                                t, uA, uB, pair_res, jc))
                    else:
                        for jc in range(8):
                            attn_v_jc(t, uA, uB, pair_res, jc)
                    normalize(t, pair_res)

        # ---- output projection (fp8 DR) + residual
        with tc.tile_pool(name="op", bufs=3, space="PSUM") as op:
            for cc in range(4):
                ps = op.tile([128, 2, 512], f32, tag="op", name=f"o{cc}")
                for ih in range(2):
                    for kc in range(4):
                        nc.tensor.matmul(
                            ps[:, ih, :],
                            lhsT=wo_sb[:, kc, ts(cc, 128)],
                            rhs=resT_sb[:, kc, ts(ih, 512)],
                            start=(kc == 0), stop=(kc == 3))
                nc.vector.tensor_add(
                    final_sb[:, cc, :], ps.rearrange("p a b -> p (a b)"),
                    final_sb[:, cc, :])
                nc.sync.dma_start(
                    out_d.rearrange("(cc p) n -> p cc n", p=128)[:, cc, :],
                    final_sb[:, cc, :])

    nc.compile()
    return nc


# ------------------------------------------------------------- SPMD dispatch
def _make_spmd_fn(nc, n_cores):
    """bass NEFF runner over axon PJRT WITHOUT buffer donation (donation
    hangs the axon backend)."""
    import jax
    import jax.core
    from jax.sharding import Mesh, PartitionSpec
    from jax.experimental.shard_map import shard_map
    from concourse import mybir
    from concourse.bass2jax import _bass_exec_p, install_neuronx_cc_hook

    install_neuronx_cc_hook()

    partition_name = nc.partition_id_tensor.name if nc.partition_id_tensor else None
    in_names, out_names, out_avals = [], [], []
    for alloc in nc.m.functions[0].allocations:
        if not isinstance(alloc, mybir.MemoryLocationSet):
            continue
        name = alloc.memorylocations[0].name
        if alloc.kind == "ExternalInput":
            if name != partition_name:
                in_names.append(name)
        elif alloc.kind == "ExternalOutput":
            out_names.append(name)
            out_avals.append(jax.core.ShapedArray(
                tuple(alloc.tensor_shape), mybir.dt.np(alloc.dtype)))

    n_params = len(in_names)
    all_in_names = list(in_names) + list(out_names)
    if partition_name is not None:
        all_in_names.append(partition_name)
    zero_outs = [np.zeros(a.shape, a.dtype) for a in out_avals]

    def _body(*args):
        operands = list(args)
        if partition_name is not None:
            from concourse.bass2jax import partition_id_tensor
            operands.append(partition_id_tensor())
        return tuple(_bass_exec_p.bind(
            *operands,
            out_avals=tuple(out_avals),
            in_names=tuple(all_in_names),
            out_names=tuple(out_names),
            lowering_input_output_aliases=(),
            sim_require_finite=True,
            sim_require_nnan=True,
            nc=nc,
        ))

    devices = jax.devices()[:n_cores]
    mesh = Mesh(np.asarray(devices), ("core",))
    sharded = jax.jit(
        shard_map(_body, mesh=mesh,
                  in_specs=(PartitionSpec("core"),) * (n_params + len(out_names)),
                  out_specs=(PartitionSpec("core"),) * len(out_names),
                  check_rep=False),
        keep_unused=True)

    def run(in_maps):
        per_core = [[np.asarray(m[k]) for k in in_names] for m in in_maps]
        concat = [np.concatenate([per_core[c][i] for c in range(n_cores)], axis=0)
                  for i in range(n_params)]
        concat += [np.concatenate([z] * n_cores, axis=0) for z in zero_outs]
        outs = [np.asarray(o) for o in sharded(*concat)]
        results = []
        for c in range(n_cores):
            m = {}
            for i, name in enumerate(out_names):
                rows = out_avals[i].shape[0]
                m[name] = outs[i][c * rows:(c + 1) * rows]
            results.append(m)
        return results

    return run


# ------------------------------------------------------------------ host prep
def _to_f8(a):
    import sys
    sys.path.insert(0, '/opt/trn_rl_repo')
    from concourse import mybir
    f8np = mybir.dt.np(mybir.dt.float8e4)
    return np.clip(a, -240.0, 240.0).astype(f8np)


def _prep_weights(w_proj, b_proj, w_out, b_out):
    # permuted qk columns: chunk m (128 cols): pair t=m//2; m even -> q, odd -> k
    # (softmax 1/8 scale now folded into the exp activation, not the weights)
    perm = np.empty(1024, np.int64)
    for m in range(8):
        t, is_k = m // 2, m % 2
        for p in range(128):
            h = 2 * t + (1 if p >= 64 else 0)
            d = p % 64
            perm[m * 128 + p] = h * 192 + 64 * is_k + d
    wqk = _to_f8(w_proj[:, perm] * W8)
    bscale = np.where((np.arange(1024) // 128) % 2 == 0, SCALE, 1.0)
    bqk = (b_proj[perm] * bscale).astype(np.float32).reshape(8, 128).T.copy()

    vperm = np.array([(j // 64) * 192 + 128 + (j % 64) for j in range(512)],
                     np.int64)
    wv = _to_f8(w_proj[:, vperm] * W8)
    bvb = np.broadcast_to(b_proj[vperm].astype(np.float32), (128, 512)).copy()

    wo = w_out.astype(bf16)
    bo = b_out.astype(np.float32).reshape(4, 128).T.copy()
    return wqk, bqk, wv, bvb, wo, bo


def kernel(x, w_proj, b_proj, w_out, b_out):
    global _cached_run
    x = np.asarray(x, np.float32)
    w_proj = np.asarray(w_proj, np.float32)
    b_proj = np.asarray(b_proj, np.float32)
    w_out = np.asarray(w_out, np.float32)
    b_out = np.asarray(b_out, np.float32)

    global _cached_nc
    if _cached_run is None:
        nc = _build_nc()
        _cached_nc = nc
        _cached_run = _make_spmd_fn(nc, B)

    wqk, bqk, wv, bvb, wo, bo = _prep_weights(w_proj, b_proj, w_out, b_out)
    in_maps = []
    for b in range(B):
        x2d = np.ascontiguousarray(x[b].reshape(C, N))
        in_maps.append(dict(
            x=x2d, xb=_to_f8(x2d), wqk=wqk, bqk=bqk,
            wv=wv, bvb=bvb, wo=wo, bo=bo))

    res = _cached_run(in_maps)
    out = np.stack([res[b]["out"].reshape(C, 32, 32) for b in range(B)])
    return out.astype(np.float32)
